# revision 48
# baseline (speedup 1.0000x reference)
"""Multi-head attention (B=2, S=2048, D=2048, H=16) on 8 Trainium2 cores.

Sharding: core = batch (2) x head-group (4 heads each). Tensor-parallel on
wq/wk/wv rows + wo columns; per-core partial outputs summed on host.

Device kernel (per core), all matmuls bf16 (fp32 PSUM accumulate):
  part 1: qT/kT (head_dim, seq) + v (seq, head_dim) projections in two
          contraction passes (dc 0..3, dc 4..15) so the PE starts before the
          x DMA completes; RoPE fused after the second pass (muls on gpsimd,
          shuffle+adds on DVE), software-pipelined one chain behind the PE so
          the DVE->gpsimd->DVE dependency line never paces the PE.
  part 2: per (q-block, head) unit u: scores k-chunk-pair matmuls into
          [128,1024] PSUM -> exp (ACT) -> attn@V; softmax denominator via
          [128,1]-output matmuls (lhsT = e chunk, rhs = ones) sharing one
          PSUM accumulation group, normalized through tiny DMA transposes +
          partition broadcasts; the output-projection chains of q-block-4-ago
          are interleaved into the stream so the PE stays fed while ACT exps.
"""

import sys

for _p in ("/opt/trn_rl_repo",):
    if _p not in sys.path:
        sys.path.insert(0, _p)

import numpy as np
import ml_dtypes

import concourse.bass as bass
import concourse.tile as tile
from concourse import bacc, mybir
from concourse.bass_utils import run_bass_kernel_spmd

F32 = mybir.dt.float32
BF16 = mybir.dt.bfloat16
NPBF = ml_dtypes.bfloat16

DIM = 2048
N_HEADS = 16
HEAD_DIM = 128
BATCH = 2
SEQ = 2048
G_HEADS = 4          # heads per core
GM = G_HEADS * HEAD_DIM  # 512 output cols per core
DC = DIM // 128      # 16 contraction chunks
DCA = 4              # pass-A contraction chunks (dc 0..3)
DCB = DC - DCA       # pass-B chunks (dc 4..15)
SC512 = SEQ // 512   # 4
SC128 = SEQ // 128   # 16
NJ2 = SC128 // 2     # 8 k-chunk pairs
NU = SC512 * G_HEADS  # 16 (q-block, head) units in part 2
INV_SQRT_HD = float(1.0 / np.sqrt(HEAD_DIM))

# even<->odd partition swap within each 32-partition group (rope pairs)
_SWAP_MASK = [i ^ 1 for i in range(32)]


def build(with_mask: bool, dbg: bool = False):
    nc = bacc.Bacc("TRN2", target_bir_lowering=False, debug=False)

    xt_d = nc.dram_tensor("xt", [DC, 128, SEQ], BF16, kind="ExternalInput").ap()
    wq_d = nc.dram_tensor("wq", [G_HEADS, 128, DC, 128], BF16, kind="ExternalInput").ap()
    wk_d = nc.dram_tensor("wk", [G_HEADS, 128, DC, 128], BF16, kind="ExternalInput").ap()
    wv_d = nc.dram_tensor("wv", [128, DC, GM], BF16, kind="ExternalInput").ap()
    wo_d = nc.dram_tensor("wo", [G_HEADS, 128, SEQ], BF16, kind="ExternalInput").ap()
    ce_d = nc.dram_tensor("ce", [128, SEQ], BF16, kind="ExternalInput").ap()
    s2_d = nc.dram_tensor("s2", [128, SEQ], BF16, kind="ExternalInput").ap()
    ones_d = nc.dram_tensor("ones", [128, 1], BF16, kind="ExternalInput").ap()
    mt_d = None
    if with_mask:
        mt_d = nc.dram_tensor("mt", [SC128, 128, SEQ], F32, kind="ExternalInput").ap()
    out_d = nc.dram_tensor("out", [SC128, 128, SEQ], F32, kind="ExternalOutput").ap()
    dbg_d = None
    if dbg:
        dbg_d = nc.dram_tensor("dbg", [4, 128, SEQ], BF16, kind="ExternalOutput").ap()

    with tile.TileContext(nc) as tc:
        persist = tc.alloc_tile_pool(name="persist", bufs=1)
        q_t = [persist.tile([128, SEQ], BF16, tag=f"q{h}", name=f"q{h}") for h in range(G_HEADS)]
        k_t = [persist.tile([128, SEQ], BF16, tag=f"k{h}", name=f"k{h}") for h in range(G_HEADS)]
        v_t = [persist.tile([128, GM], BF16, tag=f"v{s}", name=f"v{s}") for s in range(SC128)]
        o_t = [persist.tile([128, SEQ], BF16, tag=f"o{h}", name=f"o{h}") for h in range(G_HEADS)]
        wo_t = [persist.tile([128, SEQ], BF16, tag=f"wo{m}", name=f"wo{m}") for m in range(G_HEADS)]
        ce_t = persist.tile([128, SEQ], BF16, tag="ce", name="ce_t")
        s2_t = persist.tile([128, SEQ], BF16, tag="s2", name="s2_t")
        ones_t = persist.tile([128, 1], BF16, tag="ones", name="ones_t")
        warm_t = persist.tile([128, 1], F32, tag="warm", name="warm_t")

        xa_pool = tc.alloc_tile_pool(name="xa", bufs=1)
        xb_pool = tc.alloc_tile_pool(name="xb", bufs=1, side="right")
        wqk_pool = tc.alloc_tile_pool(name="wqk", bufs=2, side="right")
        rope_pool = tc.alloc_tile_pool(name="rope", bufs=3, side="right")
        ps1 = tc.alloc_tile_pool(name="ps1", bufs=4, space="PSUM")
        ps_st = tc.alloc_tile_pool(name="ps_st", bufs=2, space="PSUM", side="right")

        # ---------------- part 1, pass A: dc 0..3 ----------------
        # x tiles split into seq-halves so the first chains unblock after
        # half the startup DMA bytes
        xta0, xta1 = [], []
        with tc.high_priority():
            wt_first = wqk_pool.tile([128, DCA, 128], BF16, tag="wA", name="wtA", bufs=4)
            nc.sync.dma_start(wt_first[:], wq_d[0][:, 0:DCA, :])
            for dc in range(DCA):
                xa = xa_pool.tile([128, 1024], BF16, tag=f"xa{dc}a", name=f"xa{dc}a")
                nc.sync.dma_start(xa[:], xt_d[dc][:, 0:1024])
                xta0.append(xa)
        wt_k0 = wqk_pool.tile([128, DCA, 128], BF16, tag="wA", name="wtA", bufs=4)
        nc.sync.dma_start(wt_k0[:], wk_d[0][:, 0:DCA, :])
        for dc in range(DCA):
            xa = xa_pool.tile([128, 1024], BF16, tag=f"xa{dc}b", name=f"xa{dc}b")
            nc.sync.dma_start(xa[:], xt_d[dc][:, 1024:2048])
            xta1.append(xa)

        def xa_slice(i, start, width):
            if start < 1024:
                return xta0[i][:, start : start + width]
            return xta1[i][:, start - 1024 : start - 1024 + width]
        wt_h1 = {}
        for wd, key in ((wq_d, "q"), (wk_d, "k")):
            wt = wqk_pool.tile([128, DCA, 128], BF16, tag="wA", name="wtA", bufs=4)
            nc.sync.dma_start(wt[:], wd[1][:, 0:DCA, :])
            wt_h1[key] = wt
        nc.sync.dma_start(ones_t[:], ones_d)
        nc.scalar.activation(
            out=warm_t[:], in_=ones_t[:], func=mybir.ActivationFunctionType.Exp
        )
        wva = xa_pool.tile([128, DCA, GM], BF16, tag="wvA", name="wvA")
        nc.sync.dma_start(wva[:], wv_d[:, 0:DCA, :])

        def qk_sc_a(wt, dst, sc):
            sl = bass.ts(sc, 512)
            ps = ps1.tile([128, 512], F32, tag="ps", name="ps")
            for i in range(DCA):
                nc.tensor.matmul(
                    ps[:], wt[:, i, :], xa_slice(i, sc * 512, 512),
                    start=(i == 0), stop=(i == DCA - 1),
                )
            nc.vector.tensor_copy(out=dst[:, sl], in_=ps[:])

        def qk_chain_a(h, wd, dst, wt=None):
            if wt is None:
                wt = wqk_pool.tile([128, DCA, 128], BF16, tag="wA", name="wtA", bufs=4)
                nc.sync.dma_start(wt[:], wd[h][:, 0:DCA, :])
            for sc in range(SC512):
                qk_sc_a(wt, dst, sc)

        def v_chain_a(s):
            ps = ps1.tile([128, GM], F32, tag="ps", name="ps")
            for i in range(DCA):
                nc.tensor.matmul(
                    ps[:], xa_slice(i, s * 128, 128), wva[:, i, :],
                    start=(i == 0), stop=(i == DCA - 1),
                )
            nc.scalar.copy(out=v_t[s][:], in_=ps[:])

        xtb = []
        wvb_box = []

        def dma_xb(rng, with_wv=False):
            for dc in rng:
                xb = xb_pool.tile([128, SEQ], BF16, tag=f"xb{dc}", name=f"xb{dc}")
                nc.sync.dma_start(xb[:], xt_d[dc])
                xtb.append(xb)
            if with_wv:
                wvb = xb_pool.tile([128, DCB, GM], BF16, tag="wvB", name="wvB")
                nc.sync.dma_start(wvb[:], wv_d[:, DCA:DC, :])
                wvb_box.append(wvb)

        for h in range(G_HEADS):
            if h == 0:
                # first head: seq-halves first so chains track DMA arrivals
                for sc in (0, 1):
                    qk_sc_a(wt_first, q_t[0], sc)
                for sc in (0, 1):
                    qk_sc_a(wt_k0, k_t[0], sc)
                for sc in (2, 3):
                    qk_sc_a(wt_first, q_t[0], sc)
                for sc in (2, 3):
                    qk_sc_a(wt_k0, k_t[0], sc)
            elif h == 1:
                qk_chain_a(h, wq_d, q_t[h], wt=wt_h1["q"])
                qk_chain_a(h, wk_d, k_t[h], wt=wt_h1["k"])
            else:
                qk_chain_a(h, wq_d, q_t[h])
                qk_chain_a(h, wk_d, k_t[h])
            if h == 1:
                dma_xb(range(DCA, 10))
                for s in range(0, 8):
                    v_chain_a(s)
            if h == 2:
                # rope tables: first needed at the start of pass B
                nc.sync.dma_start(ce_t[:], ce_d)
                nc.sync.dma_start(s2_t[:], s2_d)
            if h == 3:
                dma_xb(range(10, DC), with_wv=True)
                for s in range(8, SC128):
                    v_chain_a(s)

        xa_pool.release()

        # lives from the end of part 1 through part 2
        est_pool = tc.alloc_tile_pool(name="est", bufs=10)
        if with_mask:
            mask_pool = tc.alloc_tile_pool(name="mask", bufs=4)

        e_of = {}

        def emit_st(u, jc2):
            # scores for k-chunk pair (2*jc2, 2*jc2+1) of unit u, then exp
            ic, h = divmod(u, 4)
            isl = bass.ts(ic, 512)
            st = ps_st.tile([128, 1024], F32, tag="st", name="st")
            for half in range(2):
                j = 2 * jc2 + half
                nc.tensor.matmul(
                    st[:, bass.ts(half, 512)],
                    k_t[h][:, bass.ts(j, 128)], q_t[h][:, isl],
                    start=True, stop=True,
                )
            if with_mask:
                mtl = mask_pool.tile([128, 1024], F32, tag="m", name="mtl")
                for half in range(2):
                    j = 2 * jc2 + half
                    nc.sync.dma_start(mtl[:, bass.ts(half, 512)], mt_d[j][:, isl])
                nc.vector.tensor_add(out=st[:], in0=st[:], in1=mtl[:])
            e = est_pool.tile([128, 1024], BF16, tag="e", name="e")
            nc.scalar.activation(
                out=e[:], in_=st[:], func=mybir.ActivationFunctionType.Exp
            )
            e_of[(u, jc2)] = e

        # ---------------- part 1, pass B: dc 4..15 ----------------
        # rope via swap(dst*s2) == swap(dst)*s2s with a host-swapped sin table:
        # the shuffle depends only on the PSUM add (both DVE), the two muls run
        # on gpsimd, and the final add consumes gpsimd products from well in
        # the past -- no DVE->gpsimd->DVE line paces the chain recurrence
        def rope(dst, sl):
            # shuffle on DVE (depends only on the PSUM add); everything else
            # on gpsimd so the DVE stream -- which carries the PSUM-buffer
            # WAR releases the PE waits on -- stays fast and never queues
            # behind gpsimd products
            t2 = rope_pool.tile([128, 512], BF16, tag="t2", name="t2")
            nc.vector.stream_shuffle(t2[:], dst[:, sl], _SWAP_MASK)
            t3 = rope_pool.tile([128, 512], BF16, tag="t3", name="t3")
            nc.gpsimd.tensor_mul(out=t3[:], in0=dst[:, sl], in1=ce_t[:, sl])
            t4 = rope_pool.tile([128, 512], BF16, tag="t4", name="t4")
            nc.gpsimd.tensor_mul(out=t4[:], in0=t2[:], in1=s2_t[:, sl])
            nc.gpsimd.tensor_add(out=dst[:, sl], in0=t3[:], in1=t4[:])

        def qk_chain_b(h, wd, dst):
            wt = wqk_pool.tile([128, DCB, 128], BF16, tag="wB", name="wtB")
            nc.sync.dma_start(wt[:], wd[h][:, DCA:DC, :])
            for sc in range(SC512):
                sl = bass.ts(sc, 512)
                ps = ps1.tile([128, 512], F32, tag="ps", name="ps")
                for i in range(DCB):
                    nc.tensor.matmul(
                        ps[:], wt[:, i, :], xtb[i][:, sl],
                        start=(i == 0), stop=(i == DCB - 1),
                    )
                nc.vector.tensor_add(out=dst[:, sl], in0=ps[:], in1=dst[:, sl])
                rope(dst, sl)

        def v_chain_b(s):
            wvb = wvb_box[0]
            ps = ps1.tile([128, GM], F32, tag="ps", name="ps")
            for i in range(DCB):
                nc.tensor.matmul(
                    ps[:], xtb[i][:, bass.ts(s, 128)], wvb[:, i, :],
                    start=(i == 0), stop=(i == DCB - 1),
                )
            nc.vector.tensor_add(out=v_t[s][:], in0=ps[:], in1=v_t[s][:])

        for h in range(G_HEADS):
            qk_chain_b(h, wq_d, q_t[h])
            qk_chain_b(h, wk_d, k_t[h])
            if h == 1:
                for s in range(0, 8):
                    v_chain_b(s)
            if h == 2:
                for m in range(G_HEADS):
                    nc.sync.dma_start(wo_t[m][:], wo_d[m])
            if h == 3:
                # interleave the first 4 score-pairs of unit 0 into the v tail
                # so ACT gets a head start on exp before part 2 begins
                for s in range(8, SC128):
                    v_chain_b(s)
                    if s % 2 == 1:
                        emit_st(0, (s - 9) // 2)

        ps1.release()
        rope_pool.release()
        wqk_pool.release()
        xb_pool.release()

        # ---------------- part 2: attention + output projection ----------------
        ps_ap = tc.alloc_tile_pool(name="ps_ap", bufs=2, space="PSUM")
        ps_dn = tc.alloc_tile_pool(name="ps_dn", bufs=1, space="PSUM")
        nrm_pool = tc.alloc_tile_pool(name="nrm", bufs=2)
        fout_pool = tc.alloc_tile_pool(name="fout", bufs=2)

        fout_of = {}

        def p3_chain(pu, nck, small_dma=False, st_slot=False):
            # small_dma chains run in the ACT-idle last stream/tail: the
            # PSUM->SBUF copy goes to ACT so the single pp buffer frees at
            # ACT pace instead of queueing in the busy DVE stream; st_slot
            # chains borrow the score-tile PSUM rotation once scores are done
            pic, s_loc = divmod(pu, 4)
            s_glob = pic * 4 + s_loc
            if st_slot:
                pp = ps_st.tile([128, 512], F32, tag="st", name="pp")
            else:
                pp = ps_ap.tile([128, 512], F32, tag="p3", name="pp", bufs=1)
            ssl = bass.ts(s_glob, 128)
            nsl = bass.ts(nck, 512)
            for m in range(G_HEADS):
                nc.tensor.matmul(
                    pp[:], o_t[m][:, ssl], wo_t[m][:, nsl],
                    start=(m == 0), stop=(m == G_HEADS - 1),
                )
            if small_dma:
                f = fout_pool.tile([128, 512], F32, tag="fs", name="fs", bufs=4)
                nc.scalar.copy(out=f[:], in_=pp[:])
                nc.sync.dma_start(out_d[s_glob][:, nsl], f[:])
                return
            if nck == 0:
                fout_of[pu] = fout_pool.tile([128, SEQ], F32, tag="fout", name="fout")
            f = fout_of[pu]
            nc.vector.tensor_copy(out=f[:, nsl], in_=pp[:])
            if nck == SC512 - 1:
                nc.sync.dma_start(out_d[s_glob], f[:])
                del fout_of[pu]

        def norm(u):
            ic, h = divmod(u, 4)
            isl = bass.ts(ic, 512)
            den, acc = den_acc[u]
            rsb = nrm_pool.tile([128, 4], F32, tag="rsb", name="rsb")
            nc.vector.reciprocal_approx_fast(out=rsb[:], in_=den)
            bc = nrm_pool.tile([128, 512], F32, tag="bc", name="bc")
            for qs in range(4):
                rr = nrm_pool.tile([1, 128], F32, tag=f"rr{qs}", name="rr")
                nc.sync.dma_start(rr[:], rsb[:, qs : qs + 1])
                nc.gpsimd.partition_broadcast(bc[:, bass.ts(qs, 128)], rr[:])
            nc.vector.tensor_mul(out=o_t[h][:, isl], in0=acc[:], in1=bc[:])

        den_acc = {}
        # one persistent PSUM tile for the softmax denominators; consecutive
        # units use alternating column halves so a unit's first den matmul
        # never carries a WAR on the previous unit's reciprocal (subtile deps)
        den_t = ps_dn.tile([128, 8], F32, tag="den", name="den_t")
        # output-projection chains for unit pu run during stream pu+2 (norm of
        # pu has a full stream to drain); the last two units' chains are
        # interleaved into stream 15 / the tail
        for u in range(NU):
            ic, h = divmod(u, 4)
            hsl = bass.ts(h, 128)
            acc = ps_ap.tile([128, 512], F32, tag="acc", name="acc")
            den = den_t[:, (u % 2) * 4 : (u % 2) * 4 + 4]
            den_acc[u] = (den, acc)
            last_u = u == NU - 1
            for jc2 in range(NJ2):
                if jc2 < 4:
                    emit_st(u, jc2 + 4)
                elif not last_u:
                    emit_st(u + 1, jc2 - 4)
                e = e_of.pop((u, jc2))
                for half in range(2):
                    j = 2 * jc2 + half
                    esl = e[:, bass.ts(half, 512)]
                    nc.tensor.matmul(
                        acc[:], v_t[j][:, hsl], esl,
                        start=(j == 0), stop=(j == SC128 - 1),
                    )
                    # one PSUM accumulation group for the whole [128,4] tile:
                    # the 2KB zero-region auto-starts each byte on first touch
                    for qs in range(4):
                        nc.tensor.matmul(
                            den[:, qs : qs + 1],
                            e[:, half * 512 + qs * 128 : half * 512 + (qs + 1) * 128],
                            ones_t[:],
                            start=(j == 0 and qs == 0),
                            stop=(j == SC128 - 1 and qs == 3),
                        )
                if jc2 == NJ2 - 1:
                    norm(u)
                # p3 slots start at jc2=2 (so the chain's o_t read never
                # catches the previous unit's norm-mul) and cover jc2 6-7 so
                # the steps just before a stream boundary stay long enough
                # for ACT to free the next score buffer in time
                if not last_u:
                    if jc2 in (2, 3, 6, 7) and u >= 2:
                        p3_chain(u - 2, {2: 0, 3: 1, 6: 2, 7: 3}[jc2])
                else:
                    if 2 <= jc2 <= 5:
                        p3_chain(u - 2, jc2 - 2)
                    elif jc2 >= 6:
                        p3_chain(NU - 2, jc2 - 6, small_dma=True, st_slot=True)

        # tail: remaining chains of units 14 and 15, rotating through the
        # now-idle score-tile PSUM slots to avoid single-buffer serialization
        for pu, nck, st_slot in [
            (14, 2, True), (14, 3, False),
            (15, 0, True), (15, 1, False), (15, 2, True),
        ]:
            p3_chain(pu, nck, small_dma=True, st_slot=st_slot)
        # final chain split in half so the last copy+DMA drain is shorter
        for half in range(2):
            pp = ps_ap.tile([128, 256], F32, tag="p3", name="pp", bufs=1)
            ssl = bass.ts(15, 128)
            n0 = 3 * 512 + half * 256
            for m in range(G_HEADS):
                nc.tensor.matmul(
                    pp[:], o_t[m][:, ssl], wo_t[m][:, n0 : n0 + 256],
                    start=(m == 0), stop=(m == G_HEADS - 1),
                )
            f = fout_pool.tile([128, 256], F32, tag="fh", name="fh", bufs=2)
            nc.scalar.copy(out=f[:], in_=pp[:])
            nc.sync.dma_start(out_d[15][:, n0 : n0 + 256], f[:])

        if dbg:
            nc.sync.dma_start(dbg_d[0], q_t[0][:])
            nc.sync.dma_start(dbg_d[1], k_t[0][:])
            nc.sync.dma_start(dbg_d[2], o_t[0][:])
            nc.sync.dma_start(dbg_d[3][:, 0:GM], v_t[0][:])

        ps_dn.release()
        ps_ap.release()
        ps_st.release()
        fout_pool.release()
        nrm_pool.release()
        if with_mask:
            mask_pool.release()
        est_pool.release()
        persist.release()

    nc.compile()
    return nc


_CACHE = {}


def _get_nc(with_mask: bool):
    if with_mask not in _CACHE:
        _CACHE[with_mask] = build(with_mask)
    return _CACHE[with_mask]


def kernel(in_token, freqs_cos, freqs_sin, mask, wq, wk, wv, wo):
    return _run(in_token, freqs_cos, freqs_sin, mask, wq, wk, wv, wo)


def run_traced(in_token, freqs_cos, freqs_sin, mask, wq, wk, wv, wo):
    """Test-only: run with NTFF tracing, return (output, BassKernelResults)."""
    return _run(in_token, freqs_cos, freqs_sin, mask, wq, wk, wv, wo, trace=True)


def _run(in_token, freqs_cos, freqs_sin, mask, wq, wk, wv, wo, trace=False):
    in_token = np.asarray(in_token, dtype=np.float32)
    freqs_cos = np.asarray(freqs_cos, dtype=np.float32)
    freqs_sin = np.asarray(freqs_sin, dtype=np.float32)
    mask = np.asarray(mask, dtype=np.float32)
    wq = np.asarray(wq, dtype=np.float32)
    wk = np.asarray(wk, dtype=np.float32)
    wv = np.asarray(wv, dtype=np.float32)
    wo = np.asarray(wo, dtype=np.float32)

    with_mask = bool(np.any(mask))
    nc = _get_nc(with_mask)

    # rope tables in (head_dim, seq) pair-expanded layout, signs/swap baked in
    ce = np.repeat(freqs_cos.T, 2, axis=0).astype(NPBF)  # (128, S)
    # s2 is PRE-swapped: out[p] = dst[p]*ce[p] + dst[p^1]*s2[p]
    s2 = np.empty((HEAD_DIM, SEQ), np.float32)
    s2[0::2] = -freqs_sin.T  # even out rows: x_r*c - x_i*s
    s2[1::2] = freqs_sin.T   # odd out rows:  x_i*c + x_r*s
    s2 = s2.astype(NPBF)
    ones = np.ones((128, 1), NPBF)
    if with_mask:
        mt = np.ascontiguousarray(mask.T).reshape(SC128, 128, SEQ)

    xts = [
        np.ascontiguousarray(in_token[b].T).astype(NPBF).reshape(DC, 128, SEQ)
        for b in range(BATCH)
    ]
    # per-head-group weight layouts (shared across the two batch cores)
    gmaps = []
    for g in range(G_HEADS):
        rows = slice(g * GM, (g + 1) * GM)
        # wt[h, p, dc, m] = w[g*512 + h*128 + m, dc*128 + p]
        wqt = np.ascontiguousarray(
            (wq[rows] * INV_SQRT_HD).reshape(G_HEADS, 128, DC, 128).transpose(0, 3, 2, 1)
        ).astype(NPBF)
        wkt = np.ascontiguousarray(
            wk[rows].reshape(G_HEADS, 128, DC, 128).transpose(0, 3, 2, 1)
        ).astype(NPBF)
        # wvt[p, dc, n] = wv[g*512 + n, dc*128 + p]
        wvt = np.ascontiguousarray(
            wv[rows].reshape(GM, DC, 128).transpose(2, 1, 0)
        ).astype(NPBF)
        # wot[mc, hd, n] = wo[n, g*512 + mc*128 + hd]
        wot = np.ascontiguousarray(wo[:, rows].T).astype(NPBF).reshape(G_HEADS, 128, SEQ)
        gmaps.append({"wq": wqt, "wk": wkt, "wv": wvt, "wo": wot})

    in_maps = []
    for b in range(BATCH):
        for g in range(G_HEADS):
            m = {
                "xt": xts[b], "ce": ce, "s2": s2, "ones": ones, **gmaps[g],
            }
            if with_mask:
                m["mt"] = mt
            in_maps.append(m)

    res = run_bass_kernel_spmd(nc, in_maps, core_ids=list(range(8)), trace=trace)

    out = np.zeros((BATCH, SEQ, DIM), np.float32)
    for b in range(BATCH):
        acc = None
        for g in range(G_HEADS):
            p = res.results[b * G_HEADS + g]["out"].reshape(SEQ, DIM)
            acc = p if acc is None else acc + p
        out[b] = acc
    if trace:
        return out, res
    return out


# revision 49
# speedup vs baseline: 1.0166x; 1.0166x over previous
"""Multi-head attention (B=2, S=2048, D=2048, H=16) on 8 Trainium2 cores.

Sharding: core = batch (2) x head-group (4 heads each). Tensor-parallel on
wq/wk/wv rows + wo columns; per-core partial outputs summed on host.

Device kernel (per core), all matmuls bf16 (fp32 PSUM accumulate):
  part 1: qT/kT (head_dim, seq) + v (seq, head_dim) projections in two
          contraction passes (dc 0..3, dc 4..15) so the PE starts before the
          x DMA completes; RoPE fused after the second pass (muls on gpsimd,
          shuffle+adds on DVE), software-pipelined one chain behind the PE so
          the DVE->gpsimd->DVE dependency line never paces the PE.
  part 2: per (q-block, head) unit u: scores k-chunk-pair matmuls into
          [128,1024] PSUM -> exp (ACT) -> attn@V; softmax denominator via
          [128,1]-output matmuls (lhsT = e chunk, rhs = ones) sharing one
          PSUM accumulation group, normalized through tiny DMA transposes +
          partition broadcasts; the output-projection chains of q-block-4-ago
          are interleaved into the stream so the PE stays fed while ACT exps.
"""

import sys

for _p in ("/opt/trn_rl_repo",):
    if _p not in sys.path:
        sys.path.insert(0, _p)

import numpy as np
import ml_dtypes

import concourse.bass as bass
import concourse.tile as tile
from concourse import bacc, mybir
from concourse.bass_utils import run_bass_kernel_spmd

F32 = mybir.dt.float32
BF16 = mybir.dt.bfloat16
NPBF = ml_dtypes.bfloat16

DIM = 2048
N_HEADS = 16
HEAD_DIM = 128
BATCH = 2
SEQ = 2048
G_HEADS = 4          # heads per core
GM = G_HEADS * HEAD_DIM  # 512 output cols per core
DC = DIM // 128      # 16 contraction chunks
DCA = 4              # pass-A contraction chunks (dc 0..3)
DCB = DC - DCA       # pass-B chunks (dc 4..15)
SC512 = SEQ // 512   # 4
SC128 = SEQ // 128   # 16
NJ2 = SC128 // 2     # 8 k-chunk pairs
NU = SC512 * G_HEADS  # 16 (q-block, head) units in part 2
INV_SQRT_HD = float(1.0 / np.sqrt(HEAD_DIM))

# even<->odd partition swap within each 32-partition group (rope pairs)
_SWAP_MASK = [i ^ 1 for i in range(32)]


def build(with_mask: bool, dbg: bool = False):
    nc = bacc.Bacc("TRN2", target_bir_lowering=False, debug=False)

    xt_d = nc.dram_tensor("xt", [DC, 128, SEQ], BF16, kind="ExternalInput").ap()
    wq_d = nc.dram_tensor("wq", [G_HEADS, 128, DC, 128], BF16, kind="ExternalInput").ap()
    wk_d = nc.dram_tensor("wk", [G_HEADS, 128, DC, 128], BF16, kind="ExternalInput").ap()
    wv_d = nc.dram_tensor("wv", [128, DC, GM], BF16, kind="ExternalInput").ap()
    wo_d = nc.dram_tensor("wo", [G_HEADS, 128, SEQ], BF16, kind="ExternalInput").ap()
    ce_d = nc.dram_tensor("ce", [128, SEQ], BF16, kind="ExternalInput").ap()
    s2_d = nc.dram_tensor("s2", [128, SEQ], BF16, kind="ExternalInput").ap()
    ones_d = nc.dram_tensor("ones", [128, 1], BF16, kind="ExternalInput").ap()
    mt_d = None
    if with_mask:
        mt_d = nc.dram_tensor("mt", [SC128, 128, SEQ], F32, kind="ExternalInput").ap()
    out_d = nc.dram_tensor("out", [SC128, 128, SEQ], F32, kind="ExternalOutput").ap()
    dbg_d = None
    if dbg:
        dbg_d = nc.dram_tensor("dbg", [4, 128, SEQ], BF16, kind="ExternalOutput").ap()

    with tile.TileContext(nc) as tc:
        persist = tc.alloc_tile_pool(name="persist", bufs=1)
        q_t = [persist.tile([128, SEQ], BF16, tag=f"q{h}", name=f"q{h}") for h in range(G_HEADS)]
        k_t = [persist.tile([128, SEQ], BF16, tag=f"k{h}", name=f"k{h}") for h in range(G_HEADS)]
        v_t = [persist.tile([128, GM], BF16, tag=f"v{s}", name=f"v{s}") for s in range(SC128)]
        o_t = [persist.tile([128, SEQ], BF16, tag=f"o{h}", name=f"o{h}") for h in range(G_HEADS)]
        wo_t = [persist.tile([128, SEQ], BF16, tag=f"wo{m}", name=f"wo{m}") for m in range(G_HEADS)]
        ce_t = persist.tile([128, SEQ], BF16, tag="ce", name="ce_t")
        s2_t = persist.tile([128, SEQ], BF16, tag="s2", name="s2_t")
        ones_t = persist.tile([128, 1], BF16, tag="ones", name="ones_t")
        warm_t = persist.tile([128, 1], F32, tag="warm", name="warm_t")

        xa_pool = tc.alloc_tile_pool(name="xa", bufs=1)
        xb_pool = tc.alloc_tile_pool(name="xb", bufs=1, side="right")
        wqk_pool = tc.alloc_tile_pool(name="wqk", bufs=2, side="right")
        rope_pool = tc.alloc_tile_pool(name="rope", bufs=3, side="right")
        ps1 = tc.alloc_tile_pool(name="ps1", bufs=4, space="PSUM")
        ps_st = tc.alloc_tile_pool(name="ps_st", bufs=2, space="PSUM", side="right")

        # ---------------- part 1, pass A: dc 0..3 ----------------
        # x tiles split into seq-halves so the first chains unblock after
        # half the startup DMA bytes
        xta0, xta1 = [], []
        with tc.high_priority():
            wt_first = wqk_pool.tile([128, DCA, 128], BF16, tag="wA", name="wtA", bufs=4)
            nc.sync.dma_start(wt_first[:], wq_d[0][:, 0:DCA, :])
            for dc in range(DCA):
                xa = xa_pool.tile([128, 1024], BF16, tag=f"xa{dc}a", name=f"xa{dc}a")
                nc.sync.dma_start(xa[:], xt_d[dc][:, 0:1024])
                xta0.append(xa)
        wt_k0 = wqk_pool.tile([128, DCA, 128], BF16, tag="wA", name="wtA", bufs=4)
        nc.sync.dma_start(wt_k0[:], wk_d[0][:, 0:DCA, :])
        for dc in range(DCA):
            xa = xa_pool.tile([128, 1024], BF16, tag=f"xa{dc}b", name=f"xa{dc}b")
            nc.sync.dma_start(xa[:], xt_d[dc][:, 1024:2048])
            xta1.append(xa)

        def xa_slice(i, start, width):
            if start < 1024:
                return xta0[i][:, start : start + width]
            return xta1[i][:, start - 1024 : start - 1024 + width]
        wt_h1 = {}
        for wd, key in ((wq_d, "q"), (wk_d, "k")):
            wt = wqk_pool.tile([128, DCA, 128], BF16, tag="wA", name="wtA", bufs=4)
            nc.sync.dma_start(wt[:], wd[1][:, 0:DCA, :])
            wt_h1[key] = wt
        nc.sync.dma_start(ones_t[:], ones_d)
        nc.scalar.activation(
            out=warm_t[:], in_=ones_t[:], func=mybir.ActivationFunctionType.Exp
        )
        wva = xa_pool.tile([128, DCA, GM], BF16, tag="wvA", name="wvA")
        nc.sync.dma_start(wva[:], wv_d[:, 0:DCA, :])

        def qk_sc_a(wt, dst, sc):
            sl = bass.ts(sc, 512)
            ps = ps1.tile([128, 512], F32, tag="ps", name="ps")
            for i in range(DCA):
                nc.tensor.matmul(
                    ps[:], wt[:, i, :], xa_slice(i, sc * 512, 512),
                    start=(i == 0), stop=(i == DCA - 1),
                )
            nc.vector.tensor_copy(out=dst[:, sl], in_=ps[:])

        def qk_chain_a(h, wd, dst, wt=None):
            if wt is None:
                wt = wqk_pool.tile([128, DCA, 128], BF16, tag="wA", name="wtA", bufs=4)
                nc.sync.dma_start(wt[:], wd[h][:, 0:DCA, :])
            for sc in range(SC512):
                qk_sc_a(wt, dst, sc)

        def v_chain_a(s):
            ps = ps1.tile([128, GM], F32, tag="ps", name="ps")
            for i in range(DCA):
                nc.tensor.matmul(
                    ps[:], xa_slice(i, s * 128, 128), wva[:, i, :],
                    start=(i == 0), stop=(i == DCA - 1),
                )
            nc.scalar.copy(out=v_t[s][:], in_=ps[:])

        xtb = []
        wvb_box = []

        def dma_xb(rng, with_wv=False):
            for dc in rng:
                xb = xb_pool.tile([128, SEQ], BF16, tag=f"xb{dc}", name=f"xb{dc}")
                nc.sync.dma_start(xb[:], xt_d[dc])
                xtb.append(xb)
            if with_wv:
                wvb = xb_pool.tile([128, DCB, GM], BF16, tag="wvB", name="wvB")
                nc.sync.dma_start(wvb[:], wv_d[:, DCA:DC, :])
                wvb_box.append(wvb)

        for h in range(G_HEADS):
            if h == 0:
                # first head: seq-halves first so chains track DMA arrivals
                for sc in (0, 1):
                    qk_sc_a(wt_first, q_t[0], sc)
                for sc in (0, 1):
                    qk_sc_a(wt_k0, k_t[0], sc)
                for sc in (2, 3):
                    qk_sc_a(wt_first, q_t[0], sc)
                for sc in (2, 3):
                    qk_sc_a(wt_k0, k_t[0], sc)
            elif h == 1:
                qk_chain_a(h, wq_d, q_t[h], wt=wt_h1["q"])
                qk_chain_a(h, wk_d, k_t[h], wt=wt_h1["k"])
            else:
                qk_chain_a(h, wq_d, q_t[h])
                qk_chain_a(h, wk_d, k_t[h])
            if h == 1:
                dma_xb(range(DCA, 10))
                for s in range(0, 8):
                    v_chain_a(s)
            if h == 2:
                # rope tables: first needed at the start of pass B
                nc.sync.dma_start(ce_t[:], ce_d)
                nc.sync.dma_start(s2_t[:], s2_d)
            if h == 3:
                dma_xb(range(10, DC), with_wv=True)
                for s in range(8, SC128):
                    v_chain_a(s)

        xa_pool.release()

        # lives from the end of part 1 through part 2
        est_pool = tc.alloc_tile_pool(name="est", bufs=10)
        if with_mask:
            mask_pool = tc.alloc_tile_pool(name="mask", bufs=4)

        e_of = {}

        def emit_st(u, jc2):
            # scores for k-chunk pair (2*jc2, 2*jc2+1) of unit u, then exp
            ic, h = divmod(u, 4)
            isl = bass.ts(ic, 512)
            st = ps_st.tile([128, 1024], F32, tag="st", name="st")
            for half in range(2):
                j = 2 * jc2 + half
                nc.tensor.matmul(
                    st[:, bass.ts(half, 512)],
                    k_t[h][:, bass.ts(j, 128)], q_t[h][:, isl],
                    start=True, stop=True,
                )
            if with_mask:
                mtl = mask_pool.tile([128, 1024], F32, tag="m", name="mtl")
                for half in range(2):
                    j = 2 * jc2 + half
                    nc.sync.dma_start(mtl[:, bass.ts(half, 512)], mt_d[j][:, isl])
                nc.vector.tensor_add(out=st[:], in0=st[:], in1=mtl[:])
            e = est_pool.tile([128, 1024], BF16, tag="e", name="e")
            nc.scalar.activation(
                out=e[:], in_=st[:], func=mybir.ActivationFunctionType.Exp
            )
            e_of[(u, jc2)] = e

        # ---------------- part 1, pass B: dc 4..15 ----------------
        # rope via swap(dst*s2) == swap(dst)*s2s with a host-swapped sin table:
        # the shuffle depends only on the PSUM add (both DVE), the two muls run
        # on gpsimd, and the final add consumes gpsimd products from well in
        # the past -- no DVE->gpsimd->DVE line paces the chain recurrence
        def rope(dst, sl):
            # shuffle on DVE (depends only on the PSUM add); everything else
            # on gpsimd so the DVE stream -- which carries the PSUM-buffer
            # WAR releases the PE waits on -- stays fast and never queues
            # behind gpsimd products
            t2 = rope_pool.tile([128, 512], BF16, tag="t2", name="t2")
            nc.vector.stream_shuffle(t2[:], dst[:, sl], _SWAP_MASK)
            t3 = rope_pool.tile([128, 512], BF16, tag="t3", name="t3")
            nc.gpsimd.tensor_mul(out=t3[:], in0=dst[:, sl], in1=ce_t[:, sl])
            t4 = rope_pool.tile([128, 512], BF16, tag="t4", name="t4")
            nc.gpsimd.tensor_mul(out=t4[:], in0=t2[:], in1=s2_t[:, sl])
            nc.gpsimd.tensor_add(out=dst[:, sl], in0=t3[:], in1=t4[:])

        def qk_chain_b(h, wd, dst):
            wt = wqk_pool.tile([128, DCB, 128], BF16, tag="wB", name="wtB")
            nc.sync.dma_start(wt[:], wd[h][:, DCA:DC, :])
            for sc in range(SC512):
                sl = bass.ts(sc, 512)
                ps = ps1.tile([128, 512], F32, tag="ps", name="ps")
                for i in range(DCB):
                    nc.tensor.matmul(
                        ps[:], wt[:, i, :], xtb[i][:, sl],
                        start=(i == 0), stop=(i == DCB - 1),
                    )
                nc.vector.tensor_add(out=dst[:, sl], in0=ps[:], in1=dst[:, sl])
                rope(dst, sl)

        def v_chain_b(s):
            wvb = wvb_box[0]
            ps = ps1.tile([128, GM], F32, tag="ps", name="ps")
            for i in range(DCB):
                nc.tensor.matmul(
                    ps[:], xtb[i][:, bass.ts(s, 128)], wvb[:, i, :],
                    start=(i == 0), stop=(i == DCB - 1),
                )
            nc.vector.tensor_add(out=v_t[s][:], in0=ps[:], in1=v_t[s][:])

        for h in range(G_HEADS):
            qk_chain_b(h, wq_d, q_t[h])
            qk_chain_b(h, wk_d, k_t[h])
            if h == 1:
                for s in range(0, 8):
                    v_chain_b(s)
            if h == 2:
                for m in range(G_HEADS):
                    nc.sync.dma_start(wo_t[m][:], wo_d[m])
            if h == 3:
                # interleave the first 4 score-pairs of unit 0 into the v tail
                # so ACT gets a head start on exp before part 2 begins
                for s in range(8, SC128):
                    v_chain_b(s)
                    if s % 2 == 1:
                        emit_st(0, (s - 9) // 2)

        ps1.release()
        rope_pool.release()
        wqk_pool.release()
        xb_pool.release()

        # ---------------- part 2: attention + output projection ----------------
        ps_ap = tc.alloc_tile_pool(name="ps_ap", bufs=2, space="PSUM")
        ps_dn = tc.alloc_tile_pool(name="ps_dn", bufs=1, space="PSUM")
        nrm_pool = tc.alloc_tile_pool(name="nrm", bufs=2)
        fout_pool = tc.alloc_tile_pool(name="fout", bufs=2)

        fout_of = {}

        def p3_chain(pu, nck, small_dma=False, st_slot=False):
            # small_dma chains run in the ACT-idle last stream/tail: the
            # PSUM->SBUF copy goes to ACT so the single pp buffer frees at
            # ACT pace instead of queueing in the busy DVE stream; st_slot
            # chains borrow the score-tile PSUM rotation once scores are done
            pic, s_loc = divmod(pu, 4)
            s_glob = pic * 4 + s_loc
            if st_slot:
                pp = ps_st.tile([128, 512], F32, tag="st", name="pp")
            else:
                pp = ps_ap.tile([128, 512], F32, tag="p3", name="pp", bufs=1)
            ssl = bass.ts(s_glob, 128)
            nsl = bass.ts(nck, 512)
            for m in range(G_HEADS):
                nc.tensor.matmul(
                    pp[:], o_t[m][:, ssl], wo_t[m][:, nsl],
                    start=(m == 0), stop=(m == G_HEADS - 1),
                )
            if small_dma:
                f = fout_pool.tile([128, 512], F32, tag="fs", name="fs", bufs=4)
                nc.scalar.copy(out=f[:], in_=pp[:])
                nc.sync.dma_start(out_d[s_glob][:, nsl], f[:])
                return
            if nck == 0:
                fout_of[pu] = fout_pool.tile([128, SEQ], F32, tag="fout", name="fout")
            f = fout_of[pu]
            nc.vector.tensor_copy(out=f[:, nsl], in_=pp[:])
            if nck == SC512 - 1:
                nc.sync.dma_start(out_d[s_glob], f[:])
                del fout_of[pu]

        def norm(u):
            ic, h = divmod(u, 4)
            isl = bass.ts(ic, 512)
            den, acc = den_acc[u]
            rsb = nrm_pool.tile([128, 4], F32, tag="rsb", name="rsb")
            nc.vector.reciprocal_approx_fast(out=rsb[:], in_=den)
            bc = nrm_pool.tile([128, 512], F32, tag="bc", name="bc")
            for qs in range(4):
                rr = nrm_pool.tile([1, 128], F32, tag=f"rr{qs}", name="rr")
                nc.sync.dma_start(rr[:], rsb[:, qs : qs + 1])
                nc.gpsimd.partition_broadcast(bc[:, bass.ts(qs, 128)], rr[:])
            nc.vector.tensor_mul(out=o_t[h][:, isl], in0=acc[:], in1=bc[:])

        den_acc = {}
        # one persistent PSUM tile for the softmax denominators; consecutive
        # units use alternating column halves so a unit's first den matmul
        # never carries a WAR on the previous unit's reciprocal (subtile deps)
        den_t = ps_dn.tile([128, 8], F32, tag="den", name="den_t")
        # output-projection chains for unit pu run during stream pu+2 (norm of
        # pu has a full stream to drain); the last two units' chains are
        # interleaved into stream 15 / the tail
        for u in range(NU):
            ic, h = divmod(u, 4)
            hsl = bass.ts(h, 128)
            acc = ps_ap.tile([128, 512], F32, tag="acc", name="acc")
            den = den_t[:, (u % 2) * 4 : (u % 2) * 4 + 4]
            den_acc[u] = (den, acc)
            last_u = u == NU - 1
            for jc2 in range(NJ2):
                if jc2 < 4:
                    emit_st(u, jc2 + 4)
                elif not last_u:
                    emit_st(u + 1, jc2 - 4)
                e = e_of.pop((u, jc2))
                for half in range(2):
                    j = 2 * jc2 + half
                    esl = e[:, bass.ts(half, 512)]
                    nc.tensor.matmul(
                        acc[:], v_t[j][:, hsl], esl,
                        start=(j == 0), stop=(j == SC128 - 1),
                    )
                    # one PSUM accumulation group for the whole [128,4] tile:
                    # the 2KB zero-region auto-starts each byte on first touch
                    for qs in range(4):
                        nc.tensor.matmul(
                            den[:, qs : qs + 1],
                            e[:, half * 512 + qs * 128 : half * 512 + (qs + 1) * 128],
                            ones_t[:],
                            start=(j == 0 and qs == 0),
                            stop=(j == SC128 - 1 and qs == 3),
                        )
                if jc2 == NJ2 - 1:
                    norm(u)
                # p3 slots start at jc2=2 so the chain's o_t read never
                # catches the previous unit's norm-mul still draining
                if 2 <= jc2 <= 5 and u >= 2:
                    p3_chain(u - 2, jc2 - 2)
                if last_u and jc2 >= 6:
                    p3_chain(NU - 2, jc2 - 6, small_dma=True, st_slot=True)

        # tail: remaining chains of units 14 and 15, rotating through the
        # now-idle score-tile PSUM slots to avoid single-buffer serialization
        for pu, nck, st_slot in [
            (14, 2, True), (14, 3, False),
            (15, 0, True), (15, 1, False), (15, 2, True),
        ]:
            p3_chain(pu, nck, small_dma=True, st_slot=st_slot)
        # final chain split in half so the last copy+DMA drain is shorter
        for half in range(2):
            pp = ps_ap.tile([128, 256], F32, tag="p3", name="pp", bufs=1)
            ssl = bass.ts(15, 128)
            n0 = 3 * 512 + half * 256
            for m in range(G_HEADS):
                nc.tensor.matmul(
                    pp[:], o_t[m][:, ssl], wo_t[m][:, n0 : n0 + 256],
                    start=(m == 0), stop=(m == G_HEADS - 1),
                )
            f = fout_pool.tile([128, 256], F32, tag="fh", name="fh", bufs=2)
            nc.scalar.copy(out=f[:], in_=pp[:])
            nc.sync.dma_start(out_d[15][:, n0 : n0 + 256], f[:])

        if dbg:
            nc.sync.dma_start(dbg_d[0], q_t[0][:])
            nc.sync.dma_start(dbg_d[1], k_t[0][:])
            nc.sync.dma_start(dbg_d[2], o_t[0][:])
            nc.sync.dma_start(dbg_d[3][:, 0:GM], v_t[0][:])

        ps_dn.release()
        ps_ap.release()
        ps_st.release()
        fout_pool.release()
        nrm_pool.release()
        if with_mask:
            mask_pool.release()
        est_pool.release()
        persist.release()

    nc.compile()
    return nc


_CACHE = {}


def _get_nc(with_mask: bool):
    if with_mask not in _CACHE:
        _CACHE[with_mask] = build(with_mask)
    return _CACHE[with_mask]


def kernel(in_token, freqs_cos, freqs_sin, mask, wq, wk, wv, wo):
    return _run(in_token, freqs_cos, freqs_sin, mask, wq, wk, wv, wo)


def run_traced(in_token, freqs_cos, freqs_sin, mask, wq, wk, wv, wo):
    """Test-only: run with NTFF tracing, return (output, BassKernelResults)."""
    return _run(in_token, freqs_cos, freqs_sin, mask, wq, wk, wv, wo, trace=True)


def _run(in_token, freqs_cos, freqs_sin, mask, wq, wk, wv, wo, trace=False):
    in_token = np.asarray(in_token, dtype=np.float32)
    freqs_cos = np.asarray(freqs_cos, dtype=np.float32)
    freqs_sin = np.asarray(freqs_sin, dtype=np.float32)
    mask = np.asarray(mask, dtype=np.float32)
    wq = np.asarray(wq, dtype=np.float32)
    wk = np.asarray(wk, dtype=np.float32)
    wv = np.asarray(wv, dtype=np.float32)
    wo = np.asarray(wo, dtype=np.float32)

    with_mask = bool(np.any(mask))
    nc = _get_nc(with_mask)

    # rope tables in (head_dim, seq) pair-expanded layout, signs/swap baked in
    ce = np.repeat(freqs_cos.T, 2, axis=0).astype(NPBF)  # (128, S)
    # s2 is PRE-swapped: out[p] = dst[p]*ce[p] + dst[p^1]*s2[p]
    s2 = np.empty((HEAD_DIM, SEQ), np.float32)
    s2[0::2] = -freqs_sin.T  # even out rows: x_r*c - x_i*s
    s2[1::2] = freqs_sin.T   # odd out rows:  x_i*c + x_r*s
    s2 = s2.astype(NPBF)
    ones = np.ones((128, 1), NPBF)
    if with_mask:
        mt = np.ascontiguousarray(mask.T).reshape(SC128, 128, SEQ)

    xts = [
        np.ascontiguousarray(in_token[b].T).astype(NPBF).reshape(DC, 128, SEQ)
        for b in range(BATCH)
    ]
    # per-head-group weight layouts (shared across the two batch cores)
    gmaps = []
    for g in range(G_HEADS):
        rows = slice(g * GM, (g + 1) * GM)
        # wt[h, p, dc, m] = w[g*512 + h*128 + m, dc*128 + p]
        wqt = np.ascontiguousarray(
            (wq[rows] * INV_SQRT_HD).reshape(G_HEADS, 128, DC, 128).transpose(0, 3, 2, 1)
        ).astype(NPBF)
        wkt = np.ascontiguousarray(
            wk[rows].reshape(G_HEADS, 128, DC, 128).transpose(0, 3, 2, 1)
        ).astype(NPBF)
        # wvt[p, dc, n] = wv[g*512 + n, dc*128 + p]
        wvt = np.ascontiguousarray(
            wv[rows].reshape(GM, DC, 128).transpose(2, 1, 0)
        ).astype(NPBF)
        # wot[mc, hd, n] = wo[n, g*512 + mc*128 + hd]
        wot = np.ascontiguousarray(wo[:, rows].T).astype(NPBF).reshape(G_HEADS, 128, SEQ)
        gmaps.append({"wq": wqt, "wk": wkt, "wv": wvt, "wo": wot})

    in_maps = []
    for b in range(BATCH):
        for g in range(G_HEADS):
            m = {
                "xt": xts[b], "ce": ce, "s2": s2, "ones": ones, **gmaps[g],
            }
            if with_mask:
                m["mt"] = mt
            in_maps.append(m)

    res = run_bass_kernel_spmd(nc, in_maps, core_ids=list(range(8)), trace=trace)

    out = np.zeros((BATCH, SEQ, DIM), np.float32)
    for b in range(BATCH):
        acc = None
        for g in range(G_HEADS):
            p = res.results[b * G_HEADS + g]["out"].reshape(SEQ, DIM)
            acc = p if acc is None else acc + p
        out[b] = acc
    if trace:
        return out, res
    return out
